# revision 15
# baseline (speedup 1.0000x reference)
"""Trainium2 Bass kernel for nn_Block_55336358643145 (dense transformer block).

Head-sharded attention design (v2):
- Each core owns 512 contiguous rows (of the [4096, 1024] batch-major flatten)
  for LN/projection/MLP phases, and owns 2 heads (2c, 2c+1) for attention.
- P0: LN1 + transpose of own rows -> xT, AllGather(xT) so every core has the
  full [1024, 4096] normalized input (1MB/rank).
- P1: per-head-pair Q/K/V over all 4096 positions.  V is computed
  kpos-major with per-head [ones, v] columns; the every-25th-column mask is
  folded into v/ones rows (masked kpos contribute 0 to numerator AND
  denominator), so the softmax exp needs no bias at all.
- P2: causally-exact attention (identical schedule on every core: 2 batches x
  8 q-blocks x (p+1) kpos tile-pairs) with batched bias-free exp, diagonal
  triangle handled by one bf16 multiply on ex, denominators from the ones
  column, reciprocal_approx_fast + partition_broadcast normalize.
- AllToAll redistributes attention outputs back to row-owners (1MB/rank).
- P3-P6: out-proj + residual, LN2, 4x MLP with exact Gelu (row-parallel,
  full weights, W1 prefetched during attention, W2 streamed).
All PSUM->SBUF moves run on the vector engine; biases are added with rank-1
matmuls into PSUM.  Host reassembles the 8 x [512, 1024] outputs.
"""

import contextlib

import numpy as np

import concourse.bass as bass
import concourse.tile as tile
from concourse import bacc, mybir
from concourse.bass_utils import run_bass_kernel_spmd

F32 = mybir.dt.float32
BF16 = mybir.dt.bfloat16
AF = mybir.ActivationFunctionType
ALU = mybir.AluOpType

B, S, D, H, HD, FF = 2, 2048, 1024, 16, 64, 4096
NCORE = 8
R = 512            # rows per core
DC = D // 128      # 8 d-chunks
GC = FF // 128     # 32 mlp hidden chunks
VW = 2 * (HD + 1)  # 130: per-head [ones, v(64)] twice
LN_EPS = 1e-5
JD = 25            # joined dim for the column-zero mask
QB = 256           # q-block width in attention
NP = S // QB       # 8 q-blocks per batch


def build_program(apply_ln1_gb, apply_ln2_gb):
    nc = bacc.Bacc("TRN2", target_bir_lowering=False, debug=False,
                   num_devices=NCORE)

    def inp(name, shape, dt=F32):
        return nc.dram_tensor(name, list(shape), dt, kind="ExternalInput").ap()

    io = dict(
        hs=inp("hs", (R, D)),
        wq=inp("wq", (D, D), BF16), wk=inp("wk", (D, D), BF16),
        wv=inp("wv", (D, 8 * VW), BF16), wp=inp("wp", (D, D), BF16),
        w1=inp("w1", (GC, 128, DC, 128), BF16), w2=inp("w2", (FF, D), BF16),
        bq=inp("bq", (1, D), BF16), bk=inp("bk", (1, D), BF16),
        bv=inp("bv", (1, 8 * VW), BF16), bp=inp("bp", (1, D), BF16),
        b2=inp("b2", (1, D), BF16), b1l=inp("b1l", (128, GC)),
        ln1gb=inp("ln1gb", (2, D)), ln2gb=inp("ln2gb", (2, D)),
        ident=inp("ident", (128, 128)),
        ones512=inp("ones512", (1, 512), BF16),
        vmcol=inp("vmcol", (128, 4)),
        diagm=inp("diagm", (128, 2, 2, QB), BF16),
        out=nc.dram_tensor("out", [R, D], F32, kind="ExternalOutput").ap(),
    )

    with tile.TileContext(nc) as tc:
        _build(tc, io, apply_ln1_gb, apply_ln2_gb)
    nc.compile()
    return nc


def _build(tc, io, apply_ln1_gb, apply_ln2_gb):
    nc = tc.nc

    with contextlib.ExitStack() as ctx:
        persist = ctx.enter_context(
            tc.tile_pool(name="persist", bufs=1, side="left"))
        dram = ctx.enter_context(tc.tile_pool(name="dram", bufs=1,
                                              space="DRAM"))

        # ---- constants ------------------------------------------------------
        ident_sb = persist.tile([128, 128], F32)
        nc.sync.dma_start(ident_sb[:], io["ident"][:])
        eps_sb = persist.tile([128, 1], F32)
        nc.vector.memset(eps_sb[:], LN_EPS)
        ones512 = persist.tile([1, 512], BF16)
        nc.sync.dma_start(ones512[:], io["ones512"][:])
        bq_sb = persist.tile([1, D], BF16)
        nc.sync.dma_start(bq_sb[:], io["bq"][:])
        bk_sb = persist.tile([1, D], BF16)
        nc.sync.dma_start(bk_sb[:], io["bk"][:])
        bv_sb = persist.tile([1, 8 * VW], BF16)
        nc.sync.dma_start(bv_sb[:], io["bv"][:])
        bp_sb = persist.tile([1, D], BF16)
        nc.sync.dma_start(bp_sb[:], io["bp"][:])
        b2_sb = persist.tile([1, D], BF16)
        nc.sync.dma_start(b2_sb[:], io["b2"][:])
        b1l_sb = persist.tile([128, GC], F32)
        nc.sync.dma_start(b1l_sb[:], io["b1l"][:])
        vmcol_sb = persist.tile([128, 4], F32)
        nc.sync.dma_start(vmcol_sb[:], io["vmcol"][:])
        diagm_sb = persist.tile([128, 2, 2, QB], BF16)
        nc.sync.dma_start(diagm_sb[:], io["diagm"][:])

        def ln_gb_tiles(gb_inp, nm):
            g_sb = persist.tile([128, D], F32, name=f"g_{nm}")
            b_sb = persist.tile([128, D], F32, name=f"b_{nm}")
            g_row = persist.tile([1, D], F32, name=f"gr_{nm}")
            b_row = persist.tile([1, D], F32, name=f"br_{nm}")
            nc.sync.dma_start(g_row[:], gb_inp[0:1, :])
            nc.sync.dma_start(b_row[:], gb_inp[1:2, :])
            nc.gpsimd.partition_broadcast(g_sb[:], g_row[:])
            nc.gpsimd.partition_broadcast(b_sb[:], b_row[:])
            return g_sb, b_sb

        ln1_g = ln1_b = ln2_g = ln2_b = None
        if apply_ln1_gb:
            ln1_g, ln1_b = ln_gb_tiles(io["ln1gb"], "ln1")
        if apply_ln2_gb:
            ln2_g, ln2_b = ln_gb_tiles(io["ln2gb"], "ln2")

        def layernorm(dst, src, pool, g_sb, b_sb):
            stats = pool.tile([128, 2, 6], F32, tag="ln_stats")
            sg = src.rearrange("p (g d) -> p g d", g=2)
            for g in range(2):
                nc.vector.bn_stats(out=stats[:, g, :], in_=sg[:, g, :])
            mv = pool.tile([128, 2], F32, tag="ln_mv")
            nc.vector.bn_aggr(out=mv[:], in_=stats[:])
            rstd = pool.tile([128, 1], F32, tag="ln_rstd")
            nc.scalar.activation(out=rstd[:], in_=mv[:, 1:2], func=AF.Sqrt,
                                 bias=eps_sb[:], scale=1.0)
            nc.vector.reciprocal(out=rstd[:], in_=rstd[:])
            nc.vector.tensor_scalar(out=dst, in0=src, scalar1=mv[:, 0:1],
                                    scalar2=rstd[:], op0=ALU.subtract,
                                    op1=ALU.mult)
            if g_sb is not None:
                nc.vector.tensor_mul(dst, dst, g_sb[:])
                nc.vector.tensor_add(dst, dst, b_sb[:])

        # ---- DRAM staging for collectives ----------------------------------
        KOFF, QOFF, VOFF = 0, 128 * R, 2 * 128 * R
        CHUNK = 2 * 128 * R + R * VW
        qkv_in = dram.tile([NCORE, CHUNK], BF16)
        qkv_rc = dram.tile([NCORE, CHUNK], BF16)
        ao_loc = dram.tile([NCORE, 128, R], BF16)
        ao_rc = dram.tile([NCORE, 128, R], BF16)

        # residual kept resident for P3
        hs_sb = persist.tile([128, 4, D], F32)

        # ========== P0: LN1 + transpose own rows ============================
        # ========== P1: Q/K/V for own rows, ALL heads; AllToAll =============
        es_qkv = ctx.enter_context(contextlib.ExitStack())
        qkv_pool = es_qkv.enter_context(
            tc.tile_pool(name="qkv_p", bufs=1, side="right"))
        qT_sb = qkv_pool.tile([128, B * S], BF16)
        kT_sb = qkv_pool.tile([128, B * S], BF16)
        v_sb = qkv_pool.tile([128, B * S // 128, VW], BF16)

        with tc.tile_pool(name="p0", bufs=2, side="left") as p0, \
             tc.tile_pool(name="xT_p", bufs=1, side="left") as xT_pool, \
             tc.tile_pool(name="wqkv", bufs=1, side="left") as wql, \
             tc.tile_pool(name="stg_p", bufs=3, side="left") as stg, \
             tc.tile_pool(name="v_own_p", bufs=1, side="left") as vop, \
             tc.tile_pool(name="p0ps", bufs=4, space="PSUM") as p0ps, \
             tc.tile_pool(name="qk_ps", bufs=2, space="PSUM") as qkps, \
             tc.tile_pool(name="v_ps", bufs=2, space="PSUM") as vps:
            wq_sb = wql.tile([128, DC, D], BF16)
            wk_sb = wql.tile([128, DC, D], BF16)
            wv_sb = wql.tile([128, DC, 8 * VW], BF16)
            for c in range(DC):
                nc.sync.dma_start(wq_sb[:, c, :],
                                  io["wq"][128 * c:128 * (c + 1), :])
                nc.sync.dma_start(wk_sb[:, c, :],
                                  io["wk"][128 * c:128 * (c + 1), :])
                nc.sync.dma_start(wv_sb[:, c, :],
                                  io["wv"][128 * c:128 * (c + 1), :])
            xT_own = xT_pool.tile([128, DC, R], BF16)
            for rt in range(4):
                nc.sync.dma_start(hs_sb[:, rt, :],
                                  io["hs"][128 * rt:128 * (rt + 1), :])
                xln = p0.tile([128, D], F32, tag="xln")
                layernorm(xln[:], hs_sb[:, rt, :], p0, ln1_g, ln1_b)
                for c in range(DC):
                    tp = p0ps.tile([128, 128], F32, tag="tp")
                    nc.tensor.transpose(tp[:], xln[:, 128 * c:128 * (c + 1)],
                                        ident_sb[:])
                    nc.vector.tensor_copy(
                        xT_own[:, c, 128 * rt:128 * (rt + 1)], tp[:])
            # q/k for all 8 head-pairs over own rows -> a2a chunks
            for hp in range(NCORE):
                for off, w_sb, brow in ((KOFF, wk_sb, bk_sb),
                                        (QOFF, wq_sb, bq_sb)):
                    ps = qkps.tile([128, R], F32, tag="ps")
                    for c in range(DC):
                        nc.tensor.matmul(
                            ps[:], w_sb[:, c, 128 * hp:128 * (hp + 1)],
                            xT_own[:, c, :], start=(c == 0), stop=False)
                    nc.tensor.matmul(ps[:],
                                     brow[:, 128 * hp:128 * (hp + 1)],
                                     ones512[:], start=False, stop=True)
                    st = stg.tile([128, R], BF16, tag="st")
                    nc.vector.tensor_copy(st[:], ps[:])
                    nc.sync.dma_start(
                        qkv_in[hp, off:off + 128 * R].rearrange(
                            "(p q) -> p q", p=128), st[:])
            # v (aug, all pairs) over own rows, kpos-major
            v_own = vop.tile([128, 4, 4, 2 * VW], BF16)
            for pt in range(4):
                for cg in range(4):
                    cs = slice(2 * VW * cg, 2 * VW * (cg + 1))
                    psv = vps.tile([128, 2 * VW], F32, tag="psv")
                    for c in range(DC):
                        nc.tensor.matmul(
                            psv[:], xT_own[:, c, 128 * pt:128 * (pt + 1)],
                            wv_sb[:, c, cs], start=(c == 0), stop=False)
                    nc.tensor.matmul(psv[:], ones512[0:1, 0:128],
                                     bv_sb[:, cs], start=False, stop=True)
                    nc.vector.tensor_scalar_mul(v_own[:, pt, cg, :], psv[:],
                                                vmcol_sb[:, pt:pt + 1])
            for hp in range(NCORE):
                cg, vo = hp // 2, VW * (hp % 2)
                nc.sync.dma_start(
                    qkv_in[hp, VOFF:VOFF + R * VW].rearrange(
                        "(pt p c) -> p pt c", p=128, pt=4),
                    v_own[:, :, cg, vo:vo + VW])
        nc.gpsimd.collective_compute(
            "AllToAll", ALU.bypass,
            replica_groups=[list(range(NCORE))],
            ins=[qkv_in.opt()], outs=[qkv_rc.opt()])
        with tc.tile_pool(name="asm_p", bufs=2, side="left") as asm:
            for r in range(NCORE):
                nc.sync.dma_start(
                    kT_sb[:, R * r:R * (r + 1)],
                    qkv_rc[r, KOFF:KOFF + 128 * R].rearrange(
                        "(p q) -> p q", p=128))
                nc.sync.dma_start(
                    qT_sb[:, R * r:R * (r + 1)],
                    qkv_rc[r, QOFF:QOFF + 128 * R].rearrange(
                        "(p q) -> p q", p=128))
                gt0 = 16 * (r // 4) + 4 * (r % 4)
                nc.sync.dma_start(
                    v_sb[:, gt0:gt0 + 4, :],
                    qkv_rc[r, VOFF:VOFF + R * VW].rearrange(
                        "(pt p c) -> p pt c", p=128, pt=4))

        # prefetch Wp (used in P3) during attention
        wp_pool = ctx.enter_context(contextlib.ExitStack())
        wpl = wp_pool.enter_context(
            tc.tile_pool(name="wp_p", bufs=1, side="left"))
        wp_sb = wpl.tile([128, DC, D], BF16)
        for c in range(DC):
            nc.sync.dma_start(wp_sb[:, c, :],
                              io["wp"][128 * c:128 * (c + 1), :])

        # ================= P2: attention =====================================
        es_ao = ctx.enter_context(contextlib.ExitStack())
        ao_pool = es_ao.enter_context(
            tc.tile_pool(name="ao_p", bufs=1, side="left"))
        aoraw = ao_pool.tile([64, B, 2, NP, QB], BF16)

        with tc.tile_pool(name="sc_ps", bufs=2, space="PSUM") as scps, \
             tc.tile_pool(name="oT_ps", bufs=2, space="PSUM") as otps, \
             tc.tile_pool(name="ex_p", bufs=3, side="left") as exp_pool, \
             tc.tile_pool(name="nrm_p", bufs=2, side="left") as nrm:
            for b in range(B):
                for p in range(NP):
                    # [65, 2, 512]: each head's accumulator in its own
                    # PSUM bank (start=True clears the whole bank's
                    # has_written bits, so chains must not share banks)
                    oT = otps.tile([HD + 1, 2, 512], F32, tag="oT",
                                   name=f"oT_{b}_{p}")
                    qs = slice(S * b + QB * p, S * b + QB * (p + 1))
                    for g in range(p + 1):
                        sc = scps.tile([128, 2, 2, QB], F32, tag="sc",
                                       name=f"sc_{b}_{p}_{g}")
                        for tg in range(2):
                            tf = 16 * b + 2 * g + tg
                            for h in range(2):
                                nc.tensor.matmul(
                                    sc[:, h, tg, :],
                                    kT_sb[64 * h:64 * (h + 1),
                                          128 * tf:128 * (tf + 1)],
                                    qT_sb[64 * h:64 * (h + 1), qs],
                                    start=True, stop=True)
                        ex = exp_pool.tile([128, 2, 2, QB], BF16, tag="ex",
                                           name=f"ex_{b}_{p}_{g}")
                        nc.scalar.activation(ex[:], sc[:], func=AF.Exp)
                        if g == p:
                            nc.vector.tensor_mul(ex[:], ex[:], diagm_sb[:])
                        for tg in range(2):
                            tf = 16 * b + 2 * g + tg
                            for h in range(2):
                                nc.tensor.matmul(
                                    oT[:, h, 0:QB],
                                    v_sb[:, tf,
                                         (HD + 1) * h:(HD + 1) * (h + 1)],
                                    ex[:, h, tg, :],
                                    start=(g == 0 and tg == 0),
                                    stop=(g == p and tg == 1))
                    for h in range(2):
                        nc.vector.tensor_copy(aoraw[:, b, h, p, :],
                                              oT[0:HD, h, 0:QB])
                        den0 = nrm.tile([1, QB], F32, tag="den",
                                        name=f"den_{b}_{p}_{h}")
                        nc.vector.tensor_copy(den0[:],
                                              oT[HD:HD + 1, h, 0:QB])
                        rec0 = nrm.tile([1, QB], F32, tag="rec",
                                        name=f"rec_{b}_{p}_{h}")
                        nc.vector.reciprocal_approx_fast(rec0[:], den0[:])
                        rb = nrm.tile([64, QB], F32, tag="rb",
                                      name=f"rb_{b}_{p}_{h}")
                        nc.gpsimd.partition_broadcast(rb[:], rec0[:])
                        nc.vector.tensor_mul(aoraw[:, b, h, p, :],
                                             aoraw[:, b, h, p, :], rb[:])
                dst = ao_loc[4 * b:4 * (b + 1)].rearrange(
                    "j (h d) q2 -> d h j q2", h=2)
                for h in range(2):
                    nc.sync.dma_start(
                        dst[:, h],
                        aoraw[:, b, h].rearrange("p (pj pi) q -> p pj (pi q)",
                                                 pi=2))
        es_qkv.close()  # qT/kT/v done

        nc.gpsimd.collective_compute(
            "AllToAll", ALU.bypass,
            replica_groups=[list(range(NCORE))],
            ins=[ao_loc.opt()], outs=[ao_rc.opt()])

        # ================= P3: out-proj + residual ===========================
        es_h = ctx.enter_context(contextlib.ExitStack())
        h_pool = es_h.enter_context(
            tc.tile_pool(name="h_p", bufs=1, side="right"))
        h_sb = h_pool.tile([128, 4, D], F32)
        with tc.tile_pool(name="aoT_p", bufs=1, side="left") as aotp, \
             tc.tile_pool(name="ps_wp", bufs=2, space="PSUM") as pps:
            aoT_sb = aotp.tile([128, NCORE, R], BF16)
            for r in range(NCORE):
                nc.sync.dma_start(aoT_sb[:, r, :], ao_rc[r])
            for rt in range(4):
                for cg in range(2):
                    ps = pps.tile([128, 512], F32, tag="ps",
                                  name=f"ps_wp_{rt}_{cg}")
                    for r in range(NCORE):
                        nc.tensor.matmul(
                            ps[:], aoT_sb[:, r, 128 * rt:128 * (rt + 1)],
                            wp_sb[:, r, 512 * cg:512 * (cg + 1)],
                            start=(r == 0), stop=False)
                    nc.tensor.matmul(ps[:], ones512[0:1, 0:128],
                                     bp_sb[:, 512 * cg:512 * (cg + 1)],
                                     start=False, stop=True)
                    nc.vector.tensor_add(h_sb[:, rt, 512 * cg:512 * (cg + 1)],
                                         ps[:],
                                         hs_sb[:, rt, 512 * cg:512 * (cg + 1)])
        es_ao.close()
        wp_pool.close()

        # ================= P4: LN2 + transpose ===============================
        es_mlp = ctx.enter_context(contextlib.ExitStack())
        mlp_pool = es_mlp.enter_context(
            tc.tile_pool(name="mlp_p", bufs=1, side="left"))
        h2T = mlp_pool.tile([128, DC, R], BF16)
        gT = mlp_pool.tile([128, GC, R], BF16)
        with tc.tile_pool(name="p4", bufs=2, side="left") as p4, \
             tc.tile_pool(name="p4ps", bufs=4, space="PSUM") as p4ps:
            for rt in range(4):
                h2 = p4.tile([128, D], F32, tag="h2")
                layernorm(h2[:], h_sb[:, rt, :], p4, ln2_g, ln2_b)
                for c in range(DC):
                    tp = p4ps.tile([128, 128], F32, tag="tp")
                    nc.tensor.transpose(tp[:], h2[:, 128 * c:128 * (c + 1)],
                                        ident_sb[:])
                    nc.vector.tensor_copy(
                        h2T[:, c, 128 * rt:128 * (rt + 1)], tp[:])

        # ================= P5: MLP up + gelu =================================
        with tc.tile_pool(name="w_w1", bufs=3, side="left") as w1l, \
             tc.tile_pool(name="ps_w1", bufs=2, space="PSUM") as pps:
            for gc in range(GC):
                wt = w1l.tile([128, DC, 128], BF16, tag="w1")
                nc.sync.dma_start(wt[:], io["w1"][gc])
                ps = pps.tile([128, R], F32, tag="ps", name=f"ps_w1_{gc}")
                for c in range(DC):
                    nc.tensor.matmul(ps[:], wt[:, c, :], h2T[:, c, :],
                                     start=(c == 0), stop=(c == DC - 1))
                nc.scalar.activation(gT[:, gc, :], ps[:], func=AF.Gelu,
                                     bias=b1l_sb[:, gc:gc + 1], scale=1.0)

        # ================= P6: MLP down + bias + residual ====================
        with tc.tile_pool(name="w_w2", bufs=3, side="left") as wpl2, \
             tc.tile_pool(name="o_sb", bufs=2, side="left") as osb, \
             tc.tile_pool(name="o_ps", bufs=1, space="PSUM") as pps:
            psts = [pps.tile([128, 512], F32, tag=f"o{i}", name=f"o_ps_{i}")
                    for i in range(8)]
            for gc in range(GC):
                wt = wpl2.tile([128, D], BF16, tag="w2")
                nc.sync.dma_start(wt[:], io["w2"][128 * gc:128 * (gc + 1), :])
                for qt in range(4):
                    for cg in range(2):
                        nc.tensor.matmul(
                            psts[2 * qt + cg][:],
                            gT[:, gc, 128 * qt:128 * (qt + 1)],
                            wt[:, 512 * cg:512 * (cg + 1)],
                            start=(gc == 0), stop=False)
            for qt in range(4):
                ot = osb.tile([128, D], F32, tag="ot", name=f"ot_{qt}")
                for cg in range(2):
                    nc.tensor.matmul(psts[2 * qt + cg][:],
                                     ones512[0:1, 0:128],
                                     b2_sb[:, 512 * cg:512 * (cg + 1)],
                                     start=False, stop=True)
                    nc.vector.tensor_add(ot[:, 512 * cg:512 * (cg + 1)],
                                         psts[2 * qt + cg][:],
                                         h_sb[:, qt, 512 * cg:512 * (cg + 1)])
                nc.sync.dma_start(io["out"][128 * qt:128 * (qt + 1), :], ot[:])


# ---------------------------------------------------------------------------
# Host side
# ---------------------------------------------------------------------------

_CACHE = {}
LAST_RESULT = None  # BassKernelResults of the most recent run (for test.py)


def _get_program(key):
    if key not in _CACHE:
        _CACHE[key] = build_program(*key)
    return _CACHE[key]


def kernel(hidden_states, Wq, bq, Wk, bk, Wv, bv, Wp, bp,
           ln1_g, ln1_b, ln2_g, ln2_b, W1, b1, W2, b2):
    import ml_dtypes
    f32 = lambda a: np.ascontiguousarray(np.asarray(a, dtype=np.float32))
    bf = lambda a: np.ascontiguousarray(
        np.asarray(a, dtype=np.float32).astype(ml_dtypes.bfloat16))
    hidden_states = f32(hidden_states)
    Wq, bq, Wk, bk, Wv, bv, Wp, bp = map(f32, (Wq, bq, Wk, bk, Wv, bv, Wp, bp))
    ln1_g, ln1_b, ln2_g, ln2_b = map(f32, (ln1_g, ln1_b, ln2_g, ln2_b))
    W1, b1, W2, b2 = map(f32, (W1, b1, W2, b2))

    apply_ln1 = bool(np.any(ln1_g != 1.0) or np.any(ln1_b != 0.0))
    apply_ln2 = bool(np.any(ln2_g != 1.0) or np.any(ln2_b != 0.0))
    nc = _get_program((apply_ln1, apply_ln2))

    chunk_major = lambda v: np.ascontiguousarray(v.reshape(-1, 128).T)
    pos = np.arange(S)
    keep = ((pos % JD) != (JD - 1)).astype(np.float32)
    kk = np.arange(128)[:, None]
    jj = np.arange(QB)[None, :]
    diagm = np.zeros((128, 2, 2, QB), np.float32)
    diagm[:, :, 0, :] = (kk <= jj)[:, None, :]
    diagm[:, :, 1, :] = (kk <= jj - 128)[:, None, :]

    w1x = np.ascontiguousarray(
        W1.reshape(DC, 128, GC, 128).transpose(2, 1, 0, 3))
    # Wv augmented for all 16 heads: per head [v(64), ones-slot]; the ones
    # slot gets its 1.0 from the bias row, and masked kpos rows are zeroed
    # on-device by the vmcol multiply.
    wv_aug = np.zeros((D, 8 * VW), np.float32)
    bv_aug = np.zeros((1, 8 * VW), np.float32)
    for h in range(H):
        o = (HD + 1) * h
        wv_aug[:, o:o + HD] = Wv[:, HD * h:HD * (h + 1)]
        bv_aug[0, o:o + HD] = bv[HD * h:HD * (h + 1)]
        bv_aug[0, o + HD] = 1.0
    shared = dict(
        wq=bf(Wq * 0.125), wk=bf(Wk), wv=bf(wv_aug),
        wp=bf(Wp), w1=bf(w1x), w2=bf(W2),
        bq=bf((bq * 0.125).reshape(1, D)), bk=bf(bk.reshape(1, D)),
        bv=bf(bv_aug),
        bp=bf(bp.reshape(1, D)), b2=bf(b2.reshape(1, D)),
        b1l=chunk_major(b1),
        ln1gb=np.stack([ln1_g, ln1_b]), ln2gb=np.stack([ln2_g, ln2_b]),
        ident=np.eye(128, dtype=np.float32),
        ones512=np.ones((1, 512), dtype=np.float32).astype(ml_dtypes.bfloat16),
        diagm=bf(diagm),
    )

    hs_flat = hidden_states.reshape(B * S, D)
    in_maps = []
    for core in range(NCORE):
        m = dict(shared)
        m["hs"] = np.ascontiguousarray(hs_flat[R * core:R * (core + 1)])
        own0 = 512 * (core % 4)
        m["vmcol"] = np.ascontiguousarray(
            keep[own0:own0 + 512].reshape(4, 128).T)
        in_maps.append(m)

    res = run_bass_kernel_spmd(nc, in_maps, core_ids=list(range(NCORE)))
    global LAST_RESULT
    LAST_RESULT = res

    out_full = np.empty((B * S, D), dtype=np.float32)
    for core in range(NCORE):
        out_full[R * core:R * (core + 1)] = res.results[core]["out"]
    return out_full.reshape(B, S, D)


# revision 16
# speedup vs baseline: 1.0407x; 1.0407x over previous
"""Trainium2 Bass kernel for nn_Block_55336358643145 (dense transformer block).

Head-sharded attention design (v2):
- Each core owns 512 contiguous rows (of the [4096, 1024] batch-major flatten)
  for LN/projection/MLP phases, and owns 2 heads (2c, 2c+1) for attention.
- P0: LN1 + transpose of own rows -> xT, AllGather(xT) so every core has the
  full [1024, 4096] normalized input (1MB/rank).
- P1: per-head-pair Q/K/V over all 4096 positions.  V is computed
  kpos-major with per-head [ones, v] columns; the every-25th-column mask is
  folded into v/ones rows (masked kpos contribute 0 to numerator AND
  denominator), so the softmax exp needs no bias at all.
- P2: causally-exact attention (identical schedule on every core: 2 batches x
  8 q-blocks x (p+1) kpos tile-pairs) with batched bias-free exp, diagonal
  triangle handled by one bf16 multiply on ex, denominators from the ones
  column, reciprocal_approx_fast + partition_broadcast normalize.
- AllToAll redistributes attention outputs back to row-owners (1MB/rank).
- P3-P6: out-proj + residual, LN2, 4x MLP with exact Gelu (row-parallel,
  full weights, W1 prefetched during attention, W2 streamed).
All PSUM->SBUF moves run on the vector engine; biases are added with rank-1
matmuls into PSUM.  Host reassembles the 8 x [512, 1024] outputs.
"""

import contextlib

import numpy as np

import concourse.bass as bass
import concourse.tile as tile
from concourse import bacc, mybir
from concourse.bass_utils import run_bass_kernel_spmd

F32 = mybir.dt.float32
BF16 = mybir.dt.bfloat16
AF = mybir.ActivationFunctionType
ALU = mybir.AluOpType

B, S, D, H, HD, FF = 2, 2048, 1024, 16, 64, 4096
NCORE = 8
R = 512            # rows per core
DC = D // 128      # 8 d-chunks
GC = FF // 128     # 32 mlp hidden chunks
VW = 2 * (HD + 1)  # 130: per-head [ones, v(64)] twice
LN_EPS = 1e-5
JD = 25            # joined dim for the column-zero mask
QB = 256           # q-block width in attention
NP = S // QB       # 8 q-blocks per batch


def build_program(apply_ln1_gb, apply_ln2_gb):
    nc = bacc.Bacc("TRN2", target_bir_lowering=False, debug=False,
                   num_devices=NCORE)

    def inp(name, shape, dt=F32):
        return nc.dram_tensor(name, list(shape), dt, kind="ExternalInput").ap()

    io = dict(
        hs=inp("hs", (R, D)),
        wq=inp("wq", (D, D), BF16), wk=inp("wk", (D, D), BF16),
        wv=inp("wv", (D, 8 * VW), BF16), wp=inp("wp", (D, D), BF16),
        w1=inp("w1", (GC, 128, DC, 128), BF16), w2=inp("w2", (FF, D), BF16),
        bq=inp("bq", (1, D), BF16), bk=inp("bk", (1, D), BF16),
        bv=inp("bv", (1, 8 * VW), BF16), bp=inp("bp", (1, D), BF16),
        b2=inp("b2", (1, D), BF16), b1l=inp("b1l", (128, GC)),
        ln1gb=inp("ln1gb", (2, D)), ln2gb=inp("ln2gb", (2, D)),
        ident=inp("ident", (128, 128)),
        ones512=inp("ones512", (1, 512), BF16),
        vmcol=inp("vmcol", (128, 4)),
        diagm=inp("diagm", (128, 2, 2, QB), BF16),
        out=nc.dram_tensor("out", [R, D], F32, kind="ExternalOutput").ap(),
    )

    with tile.TileContext(nc) as tc:
        _build(tc, io, apply_ln1_gb, apply_ln2_gb)
    nc.compile()
    return nc


def _build(tc, io, apply_ln1_gb, apply_ln2_gb):
    nc = tc.nc

    with contextlib.ExitStack() as ctx:
        persist = ctx.enter_context(
            tc.tile_pool(name="persist", bufs=1, side="left"))
        dram = ctx.enter_context(tc.tile_pool(name="dram", bufs=1,
                                              space="DRAM"))

        # ---- constants ------------------------------------------------------
        ident_sb = persist.tile([128, 128], F32)
        nc.sync.dma_start(ident_sb[:], io["ident"][:])
        eps_sb = persist.tile([128, 1], F32)
        nc.vector.memset(eps_sb[:], LN_EPS)
        ones512 = persist.tile([1, 512], BF16)
        nc.sync.dma_start(ones512[:], io["ones512"][:])
        bq_sb = persist.tile([1, D], BF16)
        nc.sync.dma_start(bq_sb[:], io["bq"][:])
        bk_sb = persist.tile([1, D], BF16)
        nc.sync.dma_start(bk_sb[:], io["bk"][:])
        bv_sb = persist.tile([1, 8 * VW], BF16)
        nc.sync.dma_start(bv_sb[:], io["bv"][:])
        bp_sb = persist.tile([1, D], BF16)
        nc.sync.dma_start(bp_sb[:], io["bp"][:])
        b2_sb = persist.tile([1, D], BF16)
        nc.sync.dma_start(b2_sb[:], io["b2"][:])
        b1l_sb = persist.tile([128, GC], F32)
        nc.sync.dma_start(b1l_sb[:], io["b1l"][:])
        vmcol_sb = persist.tile([128, 4], F32)
        nc.sync.dma_start(vmcol_sb[:], io["vmcol"][:])
        diagm_sb = persist.tile([128, 2, 2, QB], BF16)
        nc.sync.dma_start(diagm_sb[:], io["diagm"][:])

        def ln_gb_tiles(gb_inp, nm):
            g_sb = persist.tile([128, D], F32, name=f"g_{nm}")
            b_sb = persist.tile([128, D], F32, name=f"b_{nm}")
            g_row = persist.tile([1, D], F32, name=f"gr_{nm}")
            b_row = persist.tile([1, D], F32, name=f"br_{nm}")
            nc.sync.dma_start(g_row[:], gb_inp[0:1, :])
            nc.sync.dma_start(b_row[:], gb_inp[1:2, :])
            nc.gpsimd.partition_broadcast(g_sb[:], g_row[:])
            nc.gpsimd.partition_broadcast(b_sb[:], b_row[:])
            return g_sb, b_sb

        ln1_g = ln1_b = ln2_g = ln2_b = None
        if apply_ln1_gb:
            ln1_g, ln1_b = ln_gb_tiles(io["ln1gb"], "ln1")
        if apply_ln2_gb:
            ln2_g, ln2_b = ln_gb_tiles(io["ln2gb"], "ln2")

        def layernorm(dst, src, pool, g_sb, b_sb):
            stats = pool.tile([128, 2, 6], F32, tag="ln_stats")
            sg = src.rearrange("p (g d) -> p g d", g=2)
            for g in range(2):
                nc.vector.bn_stats(out=stats[:, g, :], in_=sg[:, g, :])
            mv = pool.tile([128, 2], F32, tag="ln_mv")
            nc.vector.bn_aggr(out=mv[:], in_=stats[:])
            rstd = pool.tile([128, 1], F32, tag="ln_rstd")
            nc.scalar.activation(out=rstd[:], in_=mv[:, 1:2], func=AF.Sqrt,
                                 bias=eps_sb[:], scale=1.0)
            nc.vector.reciprocal(out=rstd[:], in_=rstd[:])
            nc.vector.tensor_scalar(out=dst, in0=src, scalar1=mv[:, 0:1],
                                    scalar2=rstd[:], op0=ALU.subtract,
                                    op1=ALU.mult)
            if g_sb is not None:
                nc.vector.tensor_mul(dst, dst, g_sb[:])
                nc.vector.tensor_add(dst, dst, b_sb[:])

        # ---- DRAM staging for collectives ----------------------------------
        k_in = dram.tile([NCORE, 128, R], BF16)
        k_rc = dram.tile([NCORE, 128, R], BF16)
        q_in = dram.tile([NCORE, 128, R], BF16)
        q_rc = dram.tile([NCORE, 128, R], BF16)
        v_in = dram.tile([NCORE, R, VW], BF16)
        v_rc = dram.tile([NCORE, R, VW], BF16)
        ao_loc = dram.tile([NCORE, 128, R], BF16)
        ao_rc = dram.tile([NCORE, 128, R], BF16)

        # residual kept resident for P3
        hs_sb = persist.tile([128, 4, D], F32)

        # ========== P0: LN1 + transpose own rows ============================
        # ========== P1: Q/K/V for own rows, ALL heads; AllToAll =============
        es_qkv = ctx.enter_context(contextlib.ExitStack())
        qkv_pool = es_qkv.enter_context(
            tc.tile_pool(name="qkv_p", bufs=1, side="right"))
        qT_sb = qkv_pool.tile([128, B * S], BF16)
        kT_sb = qkv_pool.tile([128, B * S], BF16)
        v_sb = qkv_pool.tile([128, B * S // 128, VW], BF16)

        with tc.tile_pool(name="p0", bufs=2, side="left") as p0, \
             tc.tile_pool(name="xT_p", bufs=1, side="left") as xT_pool, \
             tc.tile_pool(name="wqkv", bufs=1, side="left") as wql, \
             tc.tile_pool(name="stg_p", bufs=3, side="left") as stg, \
             tc.tile_pool(name="v_own_p", bufs=1, side="left") as vop, \
             tc.tile_pool(name="p0ps", bufs=4, space="PSUM") as p0ps, \
             tc.tile_pool(name="qk_ps", bufs=2, space="PSUM") as qkps, \
             tc.tile_pool(name="v_ps", bufs=2, space="PSUM") as vps:
            for rt in range(4):
                nc.sync.dma_start(hs_sb[:, rt, :],
                                  io["hs"][128 * rt:128 * (rt + 1), :])
            wq_sb = wql.tile([128, DC, D], BF16)
            wk_sb = wql.tile([128, DC, D], BF16)
            wv_sb = wql.tile([128, DC, 8 * VW], BF16)
            for c in range(DC):
                nc.sync.dma_start(wk_sb[:, c, :],
                                  io["wk"][128 * c:128 * (c + 1), :])
            for c in range(DC):
                nc.sync.dma_start(wq_sb[:, c, :],
                                  io["wq"][128 * c:128 * (c + 1), :])
            for c in range(DC):
                nc.sync.dma_start(wv_sb[:, c, :],
                                  io["wv"][128 * c:128 * (c + 1), :])
            xT_own = xT_pool.tile([128, DC, R], BF16)
            for rt in range(4):
                xln = p0.tile([128, D], F32, tag="xln")
                layernorm(xln[:], hs_sb[:, rt, :], p0, ln1_g, ln1_b)
                for c in range(DC):
                    tp = p0ps.tile([128, 128], F32, tag="tp")
                    nc.tensor.transpose(tp[:], xln[:, 128 * c:128 * (c + 1)],
                                        ident_sb[:])
                    nc.vector.tensor_copy(
                        xT_own[:, c, 128 * rt:128 * (rt + 1)], tp[:])

            def qk_pass(w_sb, brow, buf_in):
                for hp in range(NCORE):
                    ps = qkps.tile([128, R], F32, tag="ps")
                    for c in range(DC):
                        nc.tensor.matmul(
                            ps[:], w_sb[:, c, 128 * hp:128 * (hp + 1)],
                            xT_own[:, c, :], start=(c == 0), stop=False)
                    nc.tensor.matmul(ps[:],
                                     brow[:, 128 * hp:128 * (hp + 1)],
                                     ones512[:], start=False, stop=True)
                    st = stg.tile([128, R], BF16, tag="st")
                    nc.vector.tensor_copy(st[:], ps[:])
                    nc.sync.dma_start(buf_in[hp], st[:])

            qk_pass(wk_sb, bk_sb, k_in)
            nc.gpsimd.collective_compute(
                "AllToAll", ALU.bypass,
                replica_groups=[list(range(NCORE))],
                ins=[k_in.opt()], outs=[k_rc.opt()])
            qk_pass(wq_sb, bq_sb, q_in)
            nc.gpsimd.collective_compute(
                "AllToAll", ALU.bypass,
                replica_groups=[list(range(NCORE))],
                ins=[q_in.opt()], outs=[q_rc.opt()])
            # v (aug, all pairs) over own rows, kpos-major
            v_own = vop.tile([128, 4, 4, 2 * VW], BF16)
            for pt in range(4):
                for cg in range(4):
                    cs = slice(2 * VW * cg, 2 * VW * (cg + 1))
                    psv = vps.tile([128, 2 * VW], F32, tag="psv")
                    for c in range(DC):
                        nc.tensor.matmul(
                            psv[:], xT_own[:, c, 128 * pt:128 * (pt + 1)],
                            wv_sb[:, c, cs], start=(c == 0), stop=False)
                    nc.tensor.matmul(psv[:], ones512[0:1, 0:128],
                                     bv_sb[:, cs], start=False, stop=True)
                    nc.vector.tensor_scalar_mul(v_own[:, pt, cg, :], psv[:],
                                                vmcol_sb[:, pt:pt + 1])
            for hp in range(NCORE):
                cg, vo = hp // 2, VW * (hp % 2)
                nc.sync.dma_start(
                    v_in[hp].rearrange("(pt p) c -> p pt c", p=128),
                    v_own[:, :, cg, vo:vo + VW])
            nc.gpsimd.collective_compute(
                "AllToAll", ALU.bypass,
                replica_groups=[list(range(NCORE))],
                ins=[v_in.opt()], outs=[v_rc.opt()])
        for r in range(NCORE):
            nc.sync.dma_start(kT_sb[:, R * r:R * (r + 1)], k_rc[r])
            nc.sync.dma_start(qT_sb[:, R * r:R * (r + 1)], q_rc[r])
            gt0 = 16 * (r // 4) + 4 * (r % 4)
            nc.sync.dma_start(
                v_sb[:, gt0:gt0 + 4, :],
                v_rc[r].rearrange("(pt p) c -> p pt c", p=128))

        # prefetch Wp (used in P3) during attention
        wp_pool = ctx.enter_context(contextlib.ExitStack())
        wpl = wp_pool.enter_context(
            tc.tile_pool(name="wp_p", bufs=1, side="left"))
        wp_sb = wpl.tile([128, DC, D], BF16)
        for c in range(DC):
            nc.sync.dma_start(wp_sb[:, c, :],
                              io["wp"][128 * c:128 * (c + 1), :])

        # ================= P2: attention =====================================
        es_ao = ctx.enter_context(contextlib.ExitStack())
        ao_pool = es_ao.enter_context(
            tc.tile_pool(name="ao_p", bufs=1, side="left"))
        aoraw = ao_pool.tile([64, B, 2, NP, QB], BF16)

        with tc.tile_pool(name="sc_ps", bufs=2, space="PSUM") as scps, \
             tc.tile_pool(name="oT_ps", bufs=2, space="PSUM") as otps, \
             tc.tile_pool(name="ex_p", bufs=3, side="left") as exp_pool, \
             tc.tile_pool(name="nrm_p", bufs=2, side="left") as nrm:
            for b in range(B):
                for p in range(NP):
                    # [65, 2, 512]: each head's accumulator in its own
                    # PSUM bank (start=True clears the whole bank's
                    # has_written bits, so chains must not share banks)
                    oT = otps.tile([HD + 1, 2, 512], F32, tag="oT",
                                   name=f"oT_{b}_{p}")
                    qs = slice(S * b + QB * p, S * b + QB * (p + 1))
                    for g in range(p + 1):
                        sc = scps.tile([128, 2, 2, QB], F32, tag="sc",
                                       name=f"sc_{b}_{p}_{g}")
                        for tg in range(2):
                            tf = 16 * b + 2 * g + tg
                            for h in range(2):
                                nc.tensor.matmul(
                                    sc[:, h, tg, :],
                                    kT_sb[64 * h:64 * (h + 1),
                                          128 * tf:128 * (tf + 1)],
                                    qT_sb[64 * h:64 * (h + 1), qs],
                                    start=True, stop=True)
                        ex = exp_pool.tile([128, 2, 2, QB], BF16, tag="ex",
                                           name=f"ex_{b}_{p}_{g}")
                        nc.scalar.activation(ex[:], sc[:], func=AF.Exp)
                        if g == p:
                            nc.vector.tensor_mul(ex[:], ex[:], diagm_sb[:])
                        for tg in range(2):
                            tf = 16 * b + 2 * g + tg
                            for h in range(2):
                                nc.tensor.matmul(
                                    oT[:, h, 0:QB],
                                    v_sb[:, tf,
                                         (HD + 1) * h:(HD + 1) * (h + 1)],
                                    ex[:, h, tg, :],
                                    start=(g == 0 and tg == 0),
                                    stop=(g == p and tg == 1))
                    for h in range(2):
                        nc.vector.tensor_copy(aoraw[:, b, h, p, :],
                                              oT[0:HD, h, 0:QB])
                        den0 = nrm.tile([1, QB], F32, tag="den",
                                        name=f"den_{b}_{p}_{h}")
                        nc.vector.tensor_copy(den0[:],
                                              oT[HD:HD + 1, h, 0:QB])
                        rec0 = nrm.tile([1, QB], F32, tag="rec",
                                        name=f"rec_{b}_{p}_{h}")
                        nc.vector.reciprocal_approx_fast(rec0[:], den0[:])
                        rb = nrm.tile([64, QB], F32, tag="rb",
                                      name=f"rb_{b}_{p}_{h}")
                        nc.gpsimd.partition_broadcast(rb[:], rec0[:])
                        nc.vector.tensor_mul(aoraw[:, b, h, p, :],
                                             aoraw[:, b, h, p, :], rb[:])
                dst = ao_loc[4 * b:4 * (b + 1)].rearrange(
                    "j (h d) q2 -> d h j q2", h=2)
                for h in range(2):
                    nc.sync.dma_start(
                        dst[:, h],
                        aoraw[:, b, h].rearrange("p (pj pi) q -> p pj (pi q)",
                                                 pi=2))
        es_qkv.close()  # qT/kT/v done

        nc.gpsimd.collective_compute(
            "AllToAll", ALU.bypass,
            replica_groups=[list(range(NCORE))],
            ins=[ao_loc.opt()], outs=[ao_rc.opt()])

        # ================= P3: out-proj + residual ===========================
        es_h = ctx.enter_context(contextlib.ExitStack())
        h_pool = es_h.enter_context(
            tc.tile_pool(name="h_p", bufs=1, side="right"))
        h_sb = h_pool.tile([128, 4, D], F32)
        with tc.tile_pool(name="aoT_p", bufs=1, side="left") as aotp, \
             tc.tile_pool(name="ps_wp", bufs=2, space="PSUM") as pps:
            aoT_sb = aotp.tile([128, NCORE, R], BF16)
            for r in range(NCORE):
                nc.sync.dma_start(aoT_sb[:, r, :], ao_rc[r])
            for rt in range(4):
                for cg in range(2):
                    ps = pps.tile([128, 512], F32, tag="ps",
                                  name=f"ps_wp_{rt}_{cg}")
                    for r in range(NCORE):
                        nc.tensor.matmul(
                            ps[:], aoT_sb[:, r, 128 * rt:128 * (rt + 1)],
                            wp_sb[:, r, 512 * cg:512 * (cg + 1)],
                            start=(r == 0), stop=False)
                    nc.tensor.matmul(ps[:], ones512[0:1, 0:128],
                                     bp_sb[:, 512 * cg:512 * (cg + 1)],
                                     start=False, stop=True)
                    nc.vector.tensor_add(h_sb[:, rt, 512 * cg:512 * (cg + 1)],
                                         ps[:],
                                         hs_sb[:, rt, 512 * cg:512 * (cg + 1)])
        es_ao.close()
        wp_pool.close()

        # ================= P4: LN2 + transpose ===============================
        es_mlp = ctx.enter_context(contextlib.ExitStack())
        mlp_pool = es_mlp.enter_context(
            tc.tile_pool(name="mlp_p", bufs=1, side="left"))
        h2T = mlp_pool.tile([128, DC, R], BF16)
        gT = mlp_pool.tile([128, GC, R], BF16)
        with tc.tile_pool(name="p4", bufs=2, side="left") as p4, \
             tc.tile_pool(name="p4ps", bufs=4, space="PSUM") as p4ps:
            for rt in range(4):
                h2 = p4.tile([128, D], F32, tag="h2")
                layernorm(h2[:], h_sb[:, rt, :], p4, ln2_g, ln2_b)
                for c in range(DC):
                    tp = p4ps.tile([128, 128], F32, tag="tp")
                    nc.tensor.transpose(tp[:], h2[:, 128 * c:128 * (c + 1)],
                                        ident_sb[:])
                    nc.vector.tensor_copy(
                        h2T[:, c, 128 * rt:128 * (rt + 1)], tp[:])

        # ================= P5: MLP up + gelu =================================
        with tc.tile_pool(name="w_w1", bufs=3, side="left") as w1l, \
             tc.tile_pool(name="ps_w1", bufs=2, space="PSUM") as pps:
            for gc in range(GC):
                wt = w1l.tile([128, DC, 128], BF16, tag="w1")
                nc.sync.dma_start(wt[:], io["w1"][gc])
                ps = pps.tile([128, R], F32, tag="ps", name=f"ps_w1_{gc}")
                for c in range(DC):
                    nc.tensor.matmul(ps[:], wt[:, c, :], h2T[:, c, :],
                                     start=(c == 0), stop=(c == DC - 1))
                nc.scalar.activation(gT[:, gc, :], ps[:], func=AF.Gelu,
                                     bias=b1l_sb[:, gc:gc + 1], scale=1.0)

        # ================= P6: MLP down + bias + residual ====================
        with tc.tile_pool(name="w_w2", bufs=3, side="left") as wpl2, \
             tc.tile_pool(name="o_sb", bufs=2, side="left") as osb, \
             tc.tile_pool(name="o_ps", bufs=1, space="PSUM") as pps:
            psts = [pps.tile([128, 512], F32, tag=f"o{i}", name=f"o_ps_{i}")
                    for i in range(8)]
            for gc in range(GC):
                wt = wpl2.tile([128, D], BF16, tag="w2")
                nc.sync.dma_start(wt[:], io["w2"][128 * gc:128 * (gc + 1), :])
                for qt in range(4):
                    for cg in range(2):
                        nc.tensor.matmul(
                            psts[2 * qt + cg][:],
                            gT[:, gc, 128 * qt:128 * (qt + 1)],
                            wt[:, 512 * cg:512 * (cg + 1)],
                            start=(gc == 0), stop=False)
            for qt in range(4):
                ot = osb.tile([128, D], F32, tag="ot", name=f"ot_{qt}")
                for cg in range(2):
                    nc.tensor.matmul(psts[2 * qt + cg][:],
                                     ones512[0:1, 0:128],
                                     b2_sb[:, 512 * cg:512 * (cg + 1)],
                                     start=False, stop=True)
                    nc.vector.tensor_add(ot[:, 512 * cg:512 * (cg + 1)],
                                         psts[2 * qt + cg][:],
                                         h_sb[:, qt, 512 * cg:512 * (cg + 1)])
                nc.sync.dma_start(io["out"][128 * qt:128 * (qt + 1), :], ot[:])


# ---------------------------------------------------------------------------
# Host side
# ---------------------------------------------------------------------------

_CACHE = {}
LAST_RESULT = None  # BassKernelResults of the most recent run (for test.py)


def _get_program(key):
    if key not in _CACHE:
        _CACHE[key] = build_program(*key)
    return _CACHE[key]


def kernel(hidden_states, Wq, bq, Wk, bk, Wv, bv, Wp, bp,
           ln1_g, ln1_b, ln2_g, ln2_b, W1, b1, W2, b2):
    import ml_dtypes
    f32 = lambda a: np.ascontiguousarray(np.asarray(a, dtype=np.float32))
    bf = lambda a: np.ascontiguousarray(
        np.asarray(a, dtype=np.float32).astype(ml_dtypes.bfloat16))
    hidden_states = f32(hidden_states)
    Wq, bq, Wk, bk, Wv, bv, Wp, bp = map(f32, (Wq, bq, Wk, bk, Wv, bv, Wp, bp))
    ln1_g, ln1_b, ln2_g, ln2_b = map(f32, (ln1_g, ln1_b, ln2_g, ln2_b))
    W1, b1, W2, b2 = map(f32, (W1, b1, W2, b2))

    apply_ln1 = bool(np.any(ln1_g != 1.0) or np.any(ln1_b != 0.0))
    apply_ln2 = bool(np.any(ln2_g != 1.0) or np.any(ln2_b != 0.0))
    nc = _get_program((apply_ln1, apply_ln2))

    chunk_major = lambda v: np.ascontiguousarray(v.reshape(-1, 128).T)
    pos = np.arange(S)
    keep = ((pos % JD) != (JD - 1)).astype(np.float32)
    kk = np.arange(128)[:, None]
    jj = np.arange(QB)[None, :]
    diagm = np.zeros((128, 2, 2, QB), np.float32)
    diagm[:, :, 0, :] = (kk <= jj)[:, None, :]
    diagm[:, :, 1, :] = (kk <= jj - 128)[:, None, :]

    w1x = np.ascontiguousarray(
        W1.reshape(DC, 128, GC, 128).transpose(2, 1, 0, 3))
    # Wv augmented for all 16 heads: per head [v(64), ones-slot]; the ones
    # slot gets its 1.0 from the bias row, and masked kpos rows are zeroed
    # on-device by the vmcol multiply.
    wv_aug = np.zeros((D, 8 * VW), np.float32)
    bv_aug = np.zeros((1, 8 * VW), np.float32)
    for h in range(H):
        o = (HD + 1) * h
        wv_aug[:, o:o + HD] = Wv[:, HD * h:HD * (h + 1)]
        bv_aug[0, o:o + HD] = bv[HD * h:HD * (h + 1)]
        bv_aug[0, o + HD] = 1.0
    shared = dict(
        wq=bf(Wq * 0.125), wk=bf(Wk), wv=bf(wv_aug),
        wp=bf(Wp), w1=bf(w1x), w2=bf(W2),
        bq=bf((bq * 0.125).reshape(1, D)), bk=bf(bk.reshape(1, D)),
        bv=bf(bv_aug),
        bp=bf(bp.reshape(1, D)), b2=bf(b2.reshape(1, D)),
        b1l=chunk_major(b1),
        ln1gb=np.stack([ln1_g, ln1_b]), ln2gb=np.stack([ln2_g, ln2_b]),
        ident=np.eye(128, dtype=np.float32),
        ones512=np.ones((1, 512), dtype=np.float32).astype(ml_dtypes.bfloat16),
        diagm=bf(diagm),
    )

    hs_flat = hidden_states.reshape(B * S, D)
    in_maps = []
    for core in range(NCORE):
        m = dict(shared)
        m["hs"] = np.ascontiguousarray(hs_flat[R * core:R * (core + 1)])
        own0 = 512 * (core % 4)
        m["vmcol"] = np.ascontiguousarray(
            keep[own0:own0 + 512].reshape(4, 128).T)
        in_maps.append(m)

    res = run_bass_kernel_spmd(nc, in_maps, core_ids=list(range(NCORE)))
    global LAST_RESULT
    LAST_RESULT = res

    out_full = np.empty((B * S, D), dtype=np.float32)
    for core in range(NCORE):
        out_full[R * core:R * (core + 1)] = res.results[core]["out"]
    return out_full.reshape(B, S, D)


# revision 18
# speedup vs baseline: 1.2022x; 1.1552x over previous
"""Trainium2 Bass kernel for nn_Block_55336358643145 (dense transformer block).

Head-sharded attention design (v2):
- Each core owns 512 contiguous rows (of the [4096, 1024] batch-major flatten)
  for LN/projection/MLP phases, and owns 2 heads (2c, 2c+1) for attention.
- P0: LN1 + transpose of own rows -> xT, AllGather(xT) so every core has the
  full [1024, 4096] normalized input (1MB/rank).
- P1: per-head-pair Q/K/V over all 4096 positions.  V is computed
  kpos-major with per-head [ones, v] columns; the every-25th-column mask is
  folded into v/ones rows (masked kpos contribute 0 to numerator AND
  denominator), so the softmax exp needs no bias at all.
- P2: causally-exact attention (identical schedule on every core: 2 batches x
  8 q-blocks x (p+1) kpos tile-pairs) with batched bias-free exp, diagonal
  triangle handled by one bf16 multiply on ex, denominators from the ones
  column, reciprocal_approx_fast + partition_broadcast normalize.
- AllToAll redistributes attention outputs back to row-owners (1MB/rank).
- P3-P6: out-proj + residual, LN2, 4x MLP with exact Gelu (row-parallel,
  full weights, W1 prefetched during attention, W2 streamed).
All PSUM->SBUF moves run on the vector engine; biases are added with rank-1
matmuls into PSUM.  Host reassembles the 8 x [512, 1024] outputs.
"""

import contextlib

import numpy as np

import concourse.bass as bass
import concourse.tile as tile
from concourse import bacc, mybir
from concourse.bass_utils import run_bass_kernel_spmd

F32 = mybir.dt.float32
FP8 = mybir.dt.float8e4
DR = mybir.MatmulPerfMode.DoubleRow
BF16 = mybir.dt.bfloat16
AF = mybir.ActivationFunctionType
ALU = mybir.AluOpType

B, S, D, H, HD, FF = 2, 2048, 1024, 16, 64, 4096
NCORE = 8
R = 512            # rows per core
DC = D // 128      # 8 d-chunks
GC = FF // 128     # 32 mlp hidden chunks
VW = 2 * (HD + 1)  # 130: per-head [ones, v(64)] twice
LN_EPS = 1e-5
JD = 25            # joined dim for the column-zero mask
QB = 256           # q-block width in attention
NP = S // QB       # 8 q-blocks per batch


def build_program(apply_ln1_gb, apply_ln2_gb, apply_bias):
    nc = bacc.Bacc("TRN2", target_bir_lowering=False, debug=False,
                   num_devices=NCORE)

    def inp(name, shape, dt=F32):
        return nc.dram_tensor(name, list(shape), dt, kind="ExternalInput").ap()

    io = dict(
        hs=inp("hs", (R, D)),
        wq=inp("wq", (D, D), BF16), wk=inp("wk", (D, D), BF16),
        wv=inp("wv", (D, 8 * VW), BF16), wp=inp("wp", (D, D), BF16),
        w1=inp("w1", (GC, 128, DC, 128), BF16), w2=inp("w2", (FF, D), FP8),
        bq=inp("bq", (1, D), BF16), bk=inp("bk", (1, D), BF16),
        bv=inp("bv", (1, 8 * VW), BF16), bp=inp("bp", (1, D), BF16),
        b2=inp("b2", (1, D), BF16), b1l=inp("b1l", (128, GC)),
        ln1gb=inp("ln1gb", (2, D)), ln2gb=inp("ln2gb", (2, D)),
        ident=inp("ident", (128, 128)),
        ones512=inp("ones512", (1, 512), BF16),
        vmcol=inp("vmcol", (128, 4)),
        diagm=inp("diagm", (128, 2, 2, QB), BF16),
        out=nc.dram_tensor("out", [R, D], F32, kind="ExternalOutput").ap(),
    )

    with tile.TileContext(nc) as tc:
        _build(tc, io, apply_ln1_gb, apply_ln2_gb, apply_bias)
    nc.compile()
    return nc


def _build(tc, io, apply_ln1_gb, apply_ln2_gb, apply_bias):
    nc = tc.nc

    with contextlib.ExitStack() as ctx:
        persist = ctx.enter_context(
            tc.tile_pool(name="persist", bufs=1, side="left"))
        dram = ctx.enter_context(tc.tile_pool(name="dram", bufs=1,
                                              space="DRAM"))

        # ---- constants ------------------------------------------------------
        ident_sb = persist.tile([128, 128], F32)
        nc.sync.dma_start(ident_sb[:], io["ident"][:])
        eps_sb = persist.tile([128, 1], F32)
        nc.vector.memset(eps_sb[:], LN_EPS)
        ones512 = persist.tile([1, 512], BF16)
        nc.sync.dma_start(ones512[:], io["ones512"][:])
        bq_sb = persist.tile([1, D], BF16)
        nc.sync.dma_start(bq_sb[:], io["bq"][:])
        bk_sb = persist.tile([1, D], BF16)
        nc.sync.dma_start(bk_sb[:], io["bk"][:])
        bv_sb = persist.tile([1, 8 * VW], BF16)
        nc.sync.dma_start(bv_sb[:], io["bv"][:])
        bp_sb = persist.tile([1, D], BF16)
        nc.sync.dma_start(bp_sb[:], io["bp"][:])
        b2_sb = persist.tile([1, D], BF16)
        nc.sync.dma_start(b2_sb[:], io["b2"][:])
        b1l_sb = persist.tile([128, GC], F32)
        nc.sync.dma_start(b1l_sb[:], io["b1l"][:])
        vmcol_sb = persist.tile([128, 4], F32)
        nc.sync.dma_start(vmcol_sb[:], io["vmcol"][:])
        diagm_sb = persist.tile([128, 2, 2, QB], BF16)
        nc.sync.dma_start(diagm_sb[:], io["diagm"][:])

        def ln_gb_tiles(gb_inp, nm):
            g_sb = persist.tile([128, D], F32, name=f"g_{nm}")
            b_sb = persist.tile([128, D], F32, name=f"b_{nm}")
            g_row = persist.tile([1, D], F32, name=f"gr_{nm}")
            b_row = persist.tile([1, D], F32, name=f"br_{nm}")
            nc.sync.dma_start(g_row[:], gb_inp[0:1, :])
            nc.sync.dma_start(b_row[:], gb_inp[1:2, :])
            nc.gpsimd.partition_broadcast(g_sb[:], g_row[:])
            nc.gpsimd.partition_broadcast(b_sb[:], b_row[:])
            return g_sb, b_sb

        ln1_g = ln1_b = ln2_g = ln2_b = None
        if apply_ln1_gb:
            ln1_g, ln1_b = ln_gb_tiles(io["ln1gb"], "ln1")
        if apply_ln2_gb:
            ln2_g, ln2_b = ln_gb_tiles(io["ln2gb"], "ln2")

        def layernorm(dst, src, pool, g_sb, b_sb):
            stats = pool.tile([128, 2, 6], F32, tag="ln_stats")
            sg = src.rearrange("p (g d) -> p g d", g=2)
            for g in range(2):
                nc.vector.bn_stats(out=stats[:, g, :], in_=sg[:, g, :])
            mv = pool.tile([128, 2], F32, tag="ln_mv")
            nc.vector.bn_aggr(out=mv[:], in_=stats[:])
            rstd = pool.tile([128, 1], F32, tag="ln_rstd")
            nc.scalar.activation(out=rstd[:], in_=mv[:, 1:2], func=AF.Sqrt,
                                 bias=eps_sb[:], scale=1.0)
            nc.vector.reciprocal(out=rstd[:], in_=rstd[:])
            nc.vector.tensor_scalar(out=dst, in0=src, scalar1=mv[:, 0:1],
                                    scalar2=rstd[:], op0=ALU.subtract,
                                    op1=ALU.mult)
            if g_sb is not None:
                nc.vector.tensor_mul(dst, dst, g_sb[:])
                nc.vector.tensor_add(dst, dst, b_sb[:])

        # ---- DRAM staging for collectives ----------------------------------
        kq_in = dram.tile([NCORE, 2, 128, R], BF16)
        kq_rc = dram.tile([NCORE, 2, 128, R], BF16)
        v_in = dram.tile([NCORE, R, VW], BF16)
        v_rc = dram.tile([NCORE, R, VW], BF16)
        ao_loc = dram.tile([NCORE, 128, R], BF16)
        ao_rc = dram.tile([NCORE, 128, R], BF16)

        # residual kept resident for P3
        hs_sb = persist.tile([128, 4, D], F32)

        # ========== P0: LN1 + transpose own rows ============================
        # ========== P1: Q/K/V for own rows, ALL heads; AllToAll =============
        es_qkv = ctx.enter_context(contextlib.ExitStack())
        qkv_pool = es_qkv.enter_context(
            tc.tile_pool(name="qkv_p", bufs=1, side="right"))
        qT_sb = qkv_pool.tile([128, B * S], BF16)
        kT_sb = qkv_pool.tile([128, B * S], BF16)
        v_sb = qkv_pool.tile([128, B * S // 128, VW], BF16)

        with tc.tile_pool(name="p0", bufs=2, side="left") as p0, \
             tc.tile_pool(name="xT_p", bufs=1, side="left") as xT_pool, \
             tc.tile_pool(name="wqkv", bufs=1, side="left") as wql, \
             tc.tile_pool(name="stg_p", bufs=3, side="left") as stg, \
             tc.tile_pool(name="v_own_p", bufs=1, side="left") as vop, \
             tc.tile_pool(name="p0ps", bufs=4, space="PSUM") as p0ps, \
             tc.tile_pool(name="qk_ps", bufs=2, space="PSUM") as qkps, \
             tc.tile_pool(name="v_ps", bufs=2, space="PSUM") as vps:
            for rt in range(4):
                nc.sync.dma_start(hs_sb[:, rt, :],
                                  io["hs"][128 * rt:128 * (rt + 1), :])
            wq_sb = wql.tile([128, DC, D], BF16)
            wk_sb = wql.tile([128, DC, D], BF16)
            wv_sb = wql.tile([128, DC, 8 * VW], BF16)
            for c in range(DC):
                nc.sync.dma_start(wv_sb[:, c, :],
                                  io["wv"][128 * c:128 * (c + 1), :])
            for c in range(DC):
                nc.sync.dma_start(wk_sb[:, c, :],
                                  io["wk"][128 * c:128 * (c + 1), :])
            for c in range(DC):
                nc.sync.dma_start(wq_sb[:, c, :],
                                  io["wq"][128 * c:128 * (c + 1), :])
            xT_own = xT_pool.tile([128, DC, R], BF16)
            for rt in range(4):
                xln = p0.tile([128, D], F32, tag="xln")
                layernorm(xln[:], hs_sb[:, rt, :], p0, ln1_g, ln1_b)
                for c in range(DC):
                    tp = p0ps.tile([128, 128], F32, tag="tp")
                    nc.tensor.transpose(tp[:], xln[:, 128 * c:128 * (c + 1)],
                                        ident_sb[:])
                    nc.vector.tensor_copy(
                        xT_own[:, c, 128 * rt:128 * (rt + 1)], tp[:])

            # v first (its a2a overlaps the k/q passes)
            v_own = vop.tile([128, 4, 4, 2 * VW], BF16)
            for pt in range(4):
                for cg in range(4):
                    cs = slice(2 * VW * cg, 2 * VW * (cg + 1))
                    psv = vps.tile([128, 2 * VW], F32, tag="psv")
                    for c in range(DC):
                        nc.tensor.matmul(
                            psv[:], xT_own[:, c, 128 * pt:128 * (pt + 1)],
                            wv_sb[:, c, cs], start=(c == 0), stop=False)
                    nc.tensor.matmul(psv[:], ones512[0:1, 0:128],
                                     bv_sb[:, cs], start=False, stop=True)
                    nc.vector.tensor_scalar_mul(v_own[:, pt, cg, :], psv[:],
                                                vmcol_sb[:, pt:pt + 1])
            for hp in range(NCORE):
                cg, vo = hp // 2, VW * (hp % 2)
                nc.sync.dma_start(
                    v_in[hp].rearrange("(pt p) c -> p pt c", p=128),
                    v_own[:, :, cg, vo:vo + VW])
            nc.gpsimd.collective_compute(
                "AllToAll", ALU.bypass,
                replica_groups=[list(range(NCORE))],
                ins=[v_in.opt()], outs=[v_rc.opt()])

            for kq, (w_sb, brow) in enumerate(((wk_sb, bk_sb),
                                               (wq_sb, bq_sb))):
                for hp in range(NCORE):
                    ps = qkps.tile([128, R], F32, tag="ps")
                    for c in range(DC):
                        nc.tensor.matmul(
                            ps[:], w_sb[:, c, 128 * hp:128 * (hp + 1)],
                            xT_own[:, c, :], start=(c == 0),
                            stop=(not apply_bias and c == DC - 1))
                    if apply_bias:
                        nc.tensor.matmul(ps[:],
                                         brow[:, 128 * hp:128 * (hp + 1)],
                                         ones512[:], start=False, stop=True)
                    st = stg.tile([128, R], BF16, tag="st")
                    nc.vector.tensor_copy(st[:], ps[:])
                    nc.sync.dma_start(kq_in[hp, kq], st[:])
            nc.gpsimd.collective_compute(
                "AllToAll", ALU.bypass,
                replica_groups=[list(range(NCORE))],
                ins=[kq_in.opt()], outs=[kq_rc.opt()])
        for r in range(NCORE):
            nc.sync.dma_start(kT_sb[:, R * r:R * (r + 1)], kq_rc[r, 0])
            nc.sync.dma_start(qT_sb[:, R * r:R * (r + 1)], kq_rc[r, 1])
            gt0 = 16 * (r // 4) + 4 * (r % 4)
            nc.sync.dma_start(
                v_sb[:, gt0:gt0 + 4, :],
                v_rc[r].rearrange("(pt p) c -> p pt c", p=128))

        # prefetch Wp (used in P3) during attention
        wp_pool = ctx.enter_context(contextlib.ExitStack())
        wpl = wp_pool.enter_context(
            tc.tile_pool(name="wp_p", bufs=1, side="left"))
        wp_sb = wpl.tile([128, DC, D], BF16)
        for c in range(DC):
            nc.sync.dma_start(wp_sb[:, c, :],
                              io["wp"][128 * c:128 * (c + 1), :])

        # ================= P2: attention =====================================
        es_ao = ctx.enter_context(contextlib.ExitStack())
        ao_pool = es_ao.enter_context(
            tc.tile_pool(name="ao_p", bufs=1, side="left"))
        aoraw = ao_pool.tile([64, B, 2, NP, QB], BF16)

        with tc.tile_pool(name="sc_ps", bufs=2, space="PSUM") as scps, \
             tc.tile_pool(name="oT_ps", bufs=2, space="PSUM") as otps, \
             tc.tile_pool(name="ex_p", bufs=3, side="left") as exp_pool, \
             tc.tile_pool(name="nrm_p", bufs=2, side="left") as nrm:
            for b in range(B):
                for p in range(NP):
                    # [65, 2, 512]: each head's accumulator in its own
                    # PSUM bank (start=True clears the whole bank's
                    # has_written bits, so chains must not share banks)
                    oT = otps.tile([HD + 1, 2, 512], F32, tag="oT",
                                   name=f"oT_{b}_{p}")
                    qs = slice(S * b + QB * p, S * b + QB * (p + 1))
                    for g in range(p + 1):
                        sc = scps.tile([128, 2, 2, QB], F32, tag="sc",
                                       name=f"sc_{b}_{p}_{g}")
                        for tg in range(2):
                            tf = 16 * b + 2 * g + tg
                            for h in range(2):
                                nc.tensor.matmul(
                                    sc[:, h, tg, :],
                                    kT_sb[64 * h:64 * (h + 1),
                                          128 * tf:128 * (tf + 1)],
                                    qT_sb[64 * h:64 * (h + 1), qs],
                                    start=True, stop=True)
                        ex = exp_pool.tile([128, 2, 2, QB], BF16, tag="ex",
                                           name=f"ex_{b}_{p}_{g}")
                        nc.scalar.activation(ex[:], sc[:], func=AF.Exp)
                        if g == p:
                            nc.vector.tensor_mul(ex[:], ex[:], diagm_sb[:])
                        for tg in range(2):
                            tf = 16 * b + 2 * g + tg
                            for h in range(2):
                                nc.tensor.matmul(
                                    oT[:, h, 0:QB],
                                    v_sb[:, tf,
                                         (HD + 1) * h:(HD + 1) * (h + 1)],
                                    ex[:, h, tg, :],
                                    start=(g == 0 and tg == 0),
                                    stop=(g == p and tg == 1))
                    for h in range(2):
                        nc.vector.tensor_copy(aoraw[:, b, h, p, :],
                                              oT[0:HD, h, 0:QB])
                        den0 = nrm.tile([1, QB], F32, tag="den",
                                        name=f"den_{b}_{p}_{h}")
                        nc.vector.tensor_copy(den0[:],
                                              oT[HD:HD + 1, h, 0:QB])
                        rec0 = nrm.tile([1, QB], F32, tag="rec",
                                        name=f"rec_{b}_{p}_{h}")
                        nc.vector.reciprocal_approx_fast(rec0[:], den0[:])
                        rb = nrm.tile([64, QB], F32, tag="rb",
                                      name=f"rb_{b}_{p}_{h}")
                        nc.gpsimd.partition_broadcast(rb[:], rec0[:])
                        nc.vector.tensor_mul(aoraw[:, b, h, p, :],
                                             aoraw[:, b, h, p, :], rb[:])
                dst = ao_loc[4 * b:4 * (b + 1)].rearrange(
                    "j (h d) q2 -> d h j q2", h=2)
                for h in range(2):
                    nc.sync.dma_start(
                        dst[:, h],
                        aoraw[:, b, h].rearrange("p (pj pi) q -> p pj (pi q)",
                                                 pi=2))
        es_qkv.close()  # qT/kT/v done

        nc.gpsimd.collective_compute(
            "AllToAll", ALU.bypass,
            replica_groups=[list(range(NCORE))],
            ins=[ao_loc.opt()], outs=[ao_rc.opt()])

        # ================= P3: out-proj + residual ===========================
        es_h = ctx.enter_context(contextlib.ExitStack())
        h_pool = es_h.enter_context(
            tc.tile_pool(name="h_p", bufs=1, side="right"))
        h_sb = h_pool.tile([128, 4, D], F32)
        with tc.tile_pool(name="aoT_p", bufs=1, side="left") as aotp, \
             tc.tile_pool(name="ps_wp", bufs=2, space="PSUM") as pps:
            aoT_sb = aotp.tile([128, NCORE, R], BF16)
            for r in range(NCORE):
                nc.sync.dma_start(aoT_sb[:, r, :], ao_rc[r])
            for rt in range(4):
                for cg in range(2):
                    ps = pps.tile([128, 512], F32, tag="ps",
                                  name=f"ps_wp_{rt}_{cg}")
                    for r in range(NCORE):
                        nc.tensor.matmul(
                            ps[:], aoT_sb[:, r, 128 * rt:128 * (rt + 1)],
                            wp_sb[:, r, 512 * cg:512 * (cg + 1)],
                            start=(r == 0),
                            stop=(not apply_bias and r == NCORE - 1))
                    if apply_bias:
                        nc.tensor.matmul(ps[:], ones512[0:1, 0:128],
                                         bp_sb[:, 512 * cg:512 * (cg + 1)],
                                         start=False, stop=True)
                    nc.vector.tensor_add(h_sb[:, rt, 512 * cg:512 * (cg + 1)],
                                         ps[:],
                                         hs_sb[:, rt, 512 * cg:512 * (cg + 1)])
        es_ao.close()
        wp_pool.close()

        # ================= P4: LN2 + transpose ===============================
        es_mlp = ctx.enter_context(contextlib.ExitStack())
        mlp_pool = es_mlp.enter_context(
            tc.tile_pool(name="mlp_p", bufs=1, side="left"))
        h2T = mlp_pool.tile([128, DC, R], BF16)
        gT = mlp_pool.tile([128, GC, R], FP8)
        with tc.tile_pool(name="p4", bufs=2, side="left") as p4, \
             tc.tile_pool(name="p4ps", bufs=4, space="PSUM") as p4ps:
            for rt in range(4):
                h2 = p4.tile([128, D], F32, tag="h2")
                layernorm(h2[:], h_sb[:, rt, :], p4, ln2_g, ln2_b)
                for c in range(DC):
                    tp = p4ps.tile([128, 128], F32, tag="tp")
                    nc.tensor.transpose(tp[:], h2[:, 128 * c:128 * (c + 1)],
                                        ident_sb[:])
                    nc.vector.tensor_copy(
                        h2T[:, c, 128 * rt:128 * (rt + 1)], tp[:])

        # ================= P5: MLP up + gelu =================================
        with tc.tile_pool(name="w_w1", bufs=3, side="left") as w1l, \
             tc.tile_pool(name="ps_w1", bufs=2, space="PSUM") as pps:
            for gc in range(GC):
                wt = w1l.tile([128, DC, 128], BF16, tag="w1")
                nc.sync.dma_start(wt[:], io["w1"][gc])
                ps = pps.tile([128, R], F32, tag="ps", name=f"ps_w1_{gc}")
                for c in range(DC):
                    nc.tensor.matmul(ps[:], wt[:, c, :], h2T[:, c, :],
                                     start=(c == 0), stop=(c == DC - 1))
                nc.scalar.activation(gT[:, gc, :], ps[:], func=AF.Gelu,
                                     bias=b1l_sb[:, gc:gc + 1], scale=1.0)

        # ================= P6: MLP down + bias + residual ====================
        with tc.tile_pool(name="w_w2", bufs=3, side="left") as wpl2, \
             tc.tile_pool(name="o_sb", bufs=2, side="left") as osb, \
             tc.tile_pool(name="o_ps", bufs=1, space="PSUM") as pps:
            psts = [pps.tile([128, 512], F32, tag=f"o{i}", name=f"o_ps_{i}")
                    for i in range(8)]
            for g2 in range(0, GC, 2):
                wt = wpl2.tile([128, 2, D], FP8, tag="w2")
                nc.sync.dma_start(
                    wt[:], io["w2"][128 * g2:128 * (g2 + 2), :].rearrange(
                        "(k p) d -> p k d", p=128))
                for qt in range(4):
                    for cg in range(2):
                        nc.tensor.matmul(
                            psts[2 * qt + cg][:],
                            gT[:, g2:g2 + 2, 128 * qt:128 * (qt + 1)],
                            wt[:, :, 512 * cg:512 * (cg + 1)],
                            start=(g2 == 0),
                            stop=(not apply_bias and g2 == GC - 2),
                            perf_mode=DR)
            for qt in range(4):
                ot = osb.tile([128, D], F32, tag="ot", name=f"ot_{qt}")
                for cg in range(2):
                    if apply_bias:
                        nc.tensor.matmul(psts[2 * qt + cg][:],
                                         ones512[0:1, 0:128],
                                         b2_sb[:, 512 * cg:512 * (cg + 1)],
                                         start=False, stop=True)
                    nc.vector.scalar_tensor_tensor(
                        ot[:, 512 * cg:512 * (cg + 1)],
                        psts[2 * qt + cg][:], 1.0 / 64.0,
                        h_sb[:, qt, 512 * cg:512 * (cg + 1)],
                        op0=ALU.mult, op1=ALU.add)
                nc.sync.dma_start(io["out"][128 * qt:128 * (qt + 1), :], ot[:])


# ---------------------------------------------------------------------------
# Host side
# ---------------------------------------------------------------------------

_CACHE = {}
LAST_RESULT = None  # BassKernelResults of the most recent run (for test.py)


def _get_program(key):
    if key not in _CACHE:
        _CACHE[key] = build_program(*key)
    return _CACHE[key]


def kernel(hidden_states, Wq, bq, Wk, bk, Wv, bv, Wp, bp,
           ln1_g, ln1_b, ln2_g, ln2_b, W1, b1, W2, b2):
    import ml_dtypes
    f32 = lambda a: np.ascontiguousarray(np.asarray(a, dtype=np.float32))
    bf = lambda a: np.ascontiguousarray(
        np.asarray(a, dtype=np.float32).astype(ml_dtypes.bfloat16))
    hidden_states = f32(hidden_states)
    Wq, bq, Wk, bk, Wv, bv, Wp, bp = map(f32, (Wq, bq, Wk, bk, Wv, bv, Wp, bp))
    ln1_g, ln1_b, ln2_g, ln2_b = map(f32, (ln1_g, ln1_b, ln2_g, ln2_b))
    W1, b1, W2, b2 = map(f32, (W1, b1, W2, b2))

    apply_ln1 = bool(np.any(ln1_g != 1.0) or np.any(ln1_b != 0.0))
    apply_ln2 = bool(np.any(ln2_g != 1.0) or np.any(ln2_b != 0.0))
    apply_bias = bool(np.any(bq != 0.0) or np.any(bk != 0.0)
                      or np.any(bp != 0.0) or np.any(b2 != 0.0))
    nc = _get_program((apply_ln1, apply_ln2, apply_bias))

    chunk_major = lambda v: np.ascontiguousarray(v.reshape(-1, 128).T)
    pos = np.arange(S)
    keep = ((pos % JD) != (JD - 1)).astype(np.float32)
    kk = np.arange(128)[:, None]
    jj = np.arange(QB)[None, :]
    diagm = np.zeros((128, 2, 2, QB), np.float32)
    diagm[:, :, 0, :] = (kk <= jj)[:, None, :]
    diagm[:, :, 1, :] = (kk <= jj - 128)[:, None, :]

    f8 = lambda a: np.ascontiguousarray(
        np.asarray(a, dtype=np.float32).astype(ml_dtypes.float8_e4m3fn))
    w1x = np.ascontiguousarray(
        W1.reshape(DC, 128, GC, 128).transpose(2, 1, 0, 3))
    # Wv augmented for all 16 heads: per head [v(64), ones-slot]; the ones
    # slot gets its 1.0 from the bias row, and masked kpos rows are zeroed
    # on-device by the vmcol multiply.
    wv_aug = np.zeros((D, 8 * VW), np.float32)
    bv_aug = np.zeros((1, 8 * VW), np.float32)
    for h in range(H):
        o = (HD + 1) * h
        wv_aug[:, o:o + HD] = Wv[:, HD * h:HD * (h + 1)]
        bv_aug[0, o:o + HD] = bv[HD * h:HD * (h + 1)]
        bv_aug[0, o + HD] = 1.0
    shared = dict(
        wq=bf(Wq * 0.125), wk=bf(Wk), wv=bf(wv_aug),
        wp=bf(Wp), w1=bf(w1x), w2=f8(W2 * 64.0),
        bq=bf((bq * 0.125).reshape(1, D)), bk=bf(bk.reshape(1, D)),
        bv=bf(bv_aug),
        bp=bf(bp.reshape(1, D)), b2=bf(64.0 * b2.reshape(1, D)),
        b1l=chunk_major(b1),
        ln1gb=np.stack([ln1_g, ln1_b]), ln2gb=np.stack([ln2_g, ln2_b]),
        ident=np.eye(128, dtype=np.float32),
        ones512=np.ones((1, 512), dtype=np.float32).astype(ml_dtypes.bfloat16),
        diagm=bf(diagm),
    )

    hs_flat = hidden_states.reshape(B * S, D)
    in_maps = []
    for core in range(NCORE):
        m = dict(shared)
        m["hs"] = np.ascontiguousarray(hs_flat[R * core:R * (core + 1)])
        own0 = 512 * (core % 4)
        m["vmcol"] = np.ascontiguousarray(
            keep[own0:own0 + 512].reshape(4, 128).T)
        in_maps.append(m)

    res = run_bass_kernel_spmd(nc, in_maps, core_ids=list(range(NCORE)))
    global LAST_RESULT
    LAST_RESULT = res

    out_full = np.empty((B * S, D), dtype=np.float32)
    for core in range(NCORE):
        out_full[R * core:R * (core + 1)] = res.results[core]["out"]
    return out_full.reshape(B, S, D)
